# revision 45
# baseline (speedup 1.0000x reference)
"""BiLSTM-CRF loss kernel for 8 Trainium2 NeuronCores.

Sharding: direction x batch. Even cores run the forward LSTM, odd cores the
backward LSTM (on host-time-reversed input). Core pair (2w, 2w+1) owns batch
window [16w, 16w+16). Each core computes its direction's partial emissions
(W_out matmul fused into the recurrence), the pair exchanges partials with one
ReduceScatter, and each core then runs the CRF (factored exp-space recurrence:
one 32x32 matmul + one elementwise multiply per step, with power-of-2
renormalization every 8 steps) plus the gold-path score (one-hot / transition
count-matrix matmuls) for 8 batches, producing nll[8].

Self-contained: hardcodes all shapes; no sibling imports.
"""

import numpy as np
import ml_dtypes

import concourse.bass as bass
import concourse.tile as tile
from concourse import mybir
from concourse.tile import add_dep_helper
from concourse.bass_utils import run_bass_kernel_spmd

F32 = mybir.dt.float32
BF16 = mybir.dt.bfloat16
I32 = mybir.dt.int32
AF = mybir.ActivationFunctionType
ALU = mybir.AluOpType

N_CORES = 8
B, T, E, H, K = 64, 256, 256, 512, 32
START, END = 30, 31
BL = 16   # batch per LSTM core
BC = 8    # batch per CRF core
LN2 = float(np.log(2.0))


# ---------------------------------------------------------------------------
# walrus-compat: this container's walrus supports only ONE sync-wait per
# instruction; Tile sometimes emits more. Split extras onto same-engine NOPs
# inserted just before the offending instruction.
# ---------------------------------------------------------------------------
def _split_multiwait(nc):
    import bass_rust
    n = 0
    for f in nc.m.functions:
        for bb in f.blocks:
            insts = bb.instructions
            if not insts:
                continue
            out = []
            changed = False
            for ins in insts:
                si = ins.sync_info
                if si is not None and si.on_wait and len(si.on_wait) > 1:
                    waits = list(si.on_wait)
                    eng = nc.engines[ins.engine]
                    for w in waits[:-1]:
                        nop = eng.nop()
                        nop_ins = nop.ins
                        cur_list = nc.cur_bb.bb.instructions
                        assert cur_list and cur_list[-1].name == nop_ins.name
                        cur_list.pop()
                        nop_ins.sync_info = bass_rust.SyncInfo(
                            on_wait=[w], on_update=[]
                        )
                        out.append(nop_ins)
                        n += 1
                    si.on_wait = [waits[-1]]
                    ins.sync_info = si
                    changed = True
                out.append(ins)
            if changed:
                bb.instructions = out
    return n


# ---------------------------------------------------------------------------
# Strip per-matmul completion increments. Every MATMUL increments the PE
# semaphore at completion and these EVT_SEM writes serialize (~26ns each), so
# the completion counter lags issue and everything waiting on "group
# complete" stalls. Keep only the increments whose cumulative value some wait
# actually targets and remap all thresholds.
# ---------------------------------------------------------------------------
def _strip_mm_incs(nc):
    blocks = [bb for f in nc.m.functions for bb in f.blocks]
    mm_sems = set()
    for bb in blocks:
        for ins in bb.instructions:
            si = ins.sync_info
            if si is None or not si.on_update:
                continue
            if type(ins).__name__ == 'InstMatmult':
                for u in si.on_update:
                    if u.update_mode == 'sem-inc':
                        mm_sems.add(u.id)
    stripped = 0
    for sem in mm_sems:
        targets = set()
        for bb in blocks:
            for ins in bb.instructions:
                si = ins.sync_info
                if si is None:
                    continue
                for w in (si.on_wait or []):
                    if w.id == sem and w.wait_mode == 'sem-ge-imm':
                        targets.add(w.wait_value)
        cum = 0
        keep_cum = []
        for bb in blocks:
            for ins in bb.instructions:
                si = ins.sync_info
                if si is None or not si.on_update:
                    continue
                ups = list(si.on_update)
                new_ups = []
                for u in ups:
                    if u.id != sem or u.update_mode != 'sem-inc':
                        new_ups.append(u)
                        continue
                    assert u.update_value == 1
                    cum += 1
                    if type(ins).__name__ == 'InstMatmult' and \
                            cum not in targets:
                        stripped += 1
                    else:
                        keep_cum.append(cum)
                        new_ups.append(u)
                if len(new_ups) != len(ups):
                    si.on_update = new_ups
                    ins.sync_info = si
        import bisect
        for bb in blocks:
            for ins in bb.instructions:
                si = ins.sync_info
                if si is None or not si.on_wait:
                    continue
                ch = False
                ws = list(si.on_wait)
                for w in ws:
                    if w.id == sem and w.wait_mode == 'sem-ge-imm':
                        nv = bisect.bisect_right(keep_cum, w.wait_value)
                        if nv != w.wait_value:
                            w.wait_value = nv
                            ch = True
                if ch:
                    si.on_wait = ws
                    ins.sync_info = si
    return stripped


# ---------------------------------------------------------------------------
# device program
# ---------------------------------------------------------------------------
def build_nc(t_steps=T, n_cores=N_CORES):
    TS = t_steps
    TB = BL * TS           # (t, b) columns per LSTM core
    BT = BC * TS           # (b, t) columns per CRF core (b-major)
    NPAIR = TS + 1         # transition pairs incl. START->t0 and tlast->END
    N_EV = (TS - 1) // 8   # renorm events

    nc = bass.Bass("TRN2", target_bir_lowering=False, debug=False,
                   num_devices=n_cores)

    # inputs (all staged per-core on host)
    xT = nc.dram_tensor("xT", [2, 128, TB], BF16, kind="ExternalInput")
    wihT = nc.dram_tensor("wihT", [2, 128, 4 * H], BF16, kind="ExternalInput")
    whhT = nc.dram_tensor("whhT", [4, 128, 4 * H], BF16, kind="ExternalInput")
    biasT = nc.dram_tensor("biasT", [128, 16], F32, kind="ExternalInput")
    woutT = nc.dram_tensor("woutT", [4, 128, K], BF16, kind="ExternalInput")
    bout = nc.dram_tensor("bout", [K, 1], F32, kind="ExternalInput")
    trans = nc.dram_tensor("trans", [K, K], F32, kind="ExternalInput")
    transT = nc.dram_tensor("transT", [K, K], F32, kind="ExternalInput")
    dirsel = nc.dram_tensor("dirsel", [K, 2], F32, kind="ExternalInput")
    tags_ext = nc.dram_tensor("tags_ext", [BC, TS + 2], F32, kind="ExternalInput")
    tags_flat = nc.dram_tensor("tags_flat", [1, BT], F32, kind="ExternalInput")
    iota_row = nc.dram_tensor("iota_row", [128, K], F32, kind="ExternalInput")
    iota_kp = nc.dram_tensor("iota_kp", [K, 1], F32, kind="ExternalInput")
    ident = nc.dram_tensor("ident", [128, 128], BF16, kind="ExternalInput")
    out = nc.dram_tensor("out", [1, BC], F32, kind="ExternalOutput")

    # collective bounce buffers
    cc_in = nc.dram_tensor("cc_in", [2 * K, BT], F32)
    cc_out = nc.dram_tensor("cc_out", [K, BT], F32)

    with tile.TileContext(nc) as tc:
        _body(tc, locals(), TS, TB, BT, NPAIR, N_EV)
    _strip_mm_incs(nc)
    return nc


def _body(tc, io, TS, TB, BT, NPAIR, N_EV):
    from contextlib import ExitStack
    nc = tc.nc
    xT, wihT, whhT, biasT, woutT = io['xT'], io['wihT'], io['whhT'], io['biasT'], io['woutT']
    bout, trans, transT, dirsel = io['bout'], io['trans'], io['transT'], io['dirsel']
    tags_ext, tags_flat, iota_row, iota_kp = io['tags_ext'], io['tags_flat'], io['iota_row'], io['iota_kp']
    ident = io['ident']
    out, cc_in, cc_out = io['out'], io['cc_in'], io['cc_out']

    with ExitStack() as top:
        persist = top.enter_context(tc.tile_pool(name="persist", bufs=1))

        # persistent tiles
        em_sb = persist.tile([K, TB], F32)           # partial emissions (t,b)
        bias_sb = persist.tile([128, 16], F32)
        nc.sync.dma_start(bias_sb[:], biasT[:, :])
        trans_sb = persist.tile([K, K], F32)
        nc.sync.dma_start(trans_sb[:], trans[:, :])
        transT_sb = persist.tile([K, K], F32)
        nc.sync.dma_start(transT_sb[:], transT[:, :])
        dirsel_sb = persist.tile([K, 2], F32)
        nc.sync.dma_start(dirsel_sb[:], dirsel[:, :])
        bout_sb = persist.tile([K, 1], F32)
        nc.sync.dma_start(bout_sb[:], bout[:, :])
        iota_row_sb = persist.tile([128, K], F32)
        nc.sync.dma_start(iota_row_sb[:], iota_row[:, :])
        iota_kp_sb = persist.tile([K, 1], F32)
        nc.sync.dma_start(iota_kp_sb[:], iota_kp[:, :])
        tagsflat_sb = persist.tile([1, BT], F32)
        nc.sync.dma_start(tagsflat_sb[:], tags_flat[:, :])
        ones32 = persist.tile([K, 1], F32)
        nc.vector.memset(ones32[:], 1.0)
        ones1x32 = persist.tile([1, K], F32)
        nc.vector.memset(ones1x32[:], 1.0)
        ident_sb = persist.tile([128, 128], BF16)
        nc.sync.dma_start(ident_sb[:], ident[:, :])

        # ---------------- phase BC pool (xg + recurrence state) -------------
        with ExitStack() as bc_stack:
            bcpool = bc_stack.enter_context(tc.tile_pool(name="bcpool", bufs=1))
            xg_sb = bcpool.tile([128, 16 * TB], BF16)

            # ---------------- phase B: Xg = W_ih_p @ X^T + bias -------------
            if True:
                bpool = bc_stack.enter_context(tc.tile_pool(name="bpool", bufs=1))
                bxpool = bc_stack.enter_context(tc.tile_pool(name="bxpool", bufs=2))
                bpsum = bc_stack.enter_context(
                    tc.tile_pool(name="bpsum", bufs=2, space="PSUM"))
                wi0 = bpool.tile([128, 4 * H], BF16)
                nc.sync.dma_start(wi0[:], wihT[0, :, :])
                wi1 = bpool.tile([128, 4 * H], BF16)
                nc.sync.dma_start(wi1[:], wihT[1, :, :])
                NCH = min(512, TB)
                NN = TB // NCH
                xs_cur = {}

                def load_x_chunk(n):
                    xs0 = bxpool.tile([128, NCH], BF16, tag="xs0")
                    nc.sync.dma_start(xs0[:], xT[0, :, n * NCH:(n + 1) * NCH])
                    xs1 = bxpool.tile([128, NCH], BF16, tag="xs1")
                    nc.sync.dma_start(xs1[:], xT[1, :, n * NCH:(n + 1) * NCH])
                    xs_cur[0], xs_cur[1] = xs0, xs1

                def emit_xg_unit(j, n):
                    if j == 0:
                        load_x_chunk(n)
                    ps = bpsum.tile([128, NCH], F32, tag="bps")
                    nc.tensor.matmul(ps[:], wi0[:, j * 128:(j + 1) * 128],
                                     xs_cur[0][:], start=True, stop=False)
                    nc.tensor.matmul(ps[:], wi1[:, j * 128:(j + 1) * 128],
                                     xs_cur[1][:], start=False, stop=True)
                    dst = xg_sb[:, j * TB + n * NCH: j * TB + (n + 1) * NCH]
                    if (j + n) % 2 == 0:
                        nc.scalar.activation(dst, ps[:], AF.Identity,
                                             bias=bias_sb[:, j:j + 1])
                    else:
                        nc.vector.tensor_scalar_add(dst, ps[:],
                                                    bias_sb[:, j:j + 1])

                # prologue: first t-chunk (n=0) of Xg for every j; the rest is
                # interleaved into the recurrence to fill PE stalls
                xg_work = []
                for n in range(NN):
                    for j in range(16):
                        if n == 0:
                            emit_xg_unit(j, n)
                        else:
                            xg_work.append((j, n))

            # ---------------- phase C: recurrence + fused emissions ---------
            with ExitStack() as c_stack:
                cpool = c_stack.enter_context(tc.tile_pool(name="cpool", bufs=1))
                whh_sb = cpool.tile([128, 4 * 4 * H], BF16)
                for c in range(4):
                    nc.sync.dma_start(
                        whh_sb[:, c * 4 * H:(c + 1) * 4 * H], whhT[c, :, :])
                wout_sb = cpool.tile([128, 4 * K], BF16)
                for c in range(4):
                    nc.sync.dma_start(wout_sb[:, c * K:(c + 1) * K],
                                      woutT[c, :, :])

                spool = c_stack.enter_context(tc.tile_pool(name="spool", bufs=2))
                qpool = c_stack.enter_context(tc.tile_pool(name="qpool", bufs=3))
                gpsum = c_stack.enter_context(
                    tc.tile_pool(name="gpsum", bufs=1, space="PSUM"))
                empsum = c_stack.enter_context(
                    tc.tile_pool(name="empsum", bufs=2, space="PSUM"))

                # single stream, half-split h/c; Xg preloaded into PSUM via
                # identity matmul so gates = PSUM directly (no DVE add)
                h_prev = spool.tile([128, 64], BF16, tag="h")
                nc.vector.memset(h_prev[:], 0.0)
                c_prev = spool.tile([128, 64], F32, tag="c")
                nc.vector.memset(c_prev[:], 0.0)

                xg_v = xg_sb[:].rearrange("p (j t b) -> p j t b", j=16, t=TS)
                em_copies = []
                for t in range(TS):
                    # two half acc-groups; gate row order g,i,f,o lets the
                    # first half's tanh(g)/sig(i)/i*g overlap the second
                    # half's matmuls
                    gh = []
                    for grp in range(2):
                        gps = gpsum.tile([128, 128], F32,
                                         tag=f"g{grp}_{t % 2}", name="gps")
                        nc.tensor.matmul(
                            gps[:], ident_sb[:],
                            xg_v[:, 8 * grp:8 * grp + 8, t, :],
                            start=True, stop=False)
                        for c_in in range(4):
                            for j in range(8 * grp, 8 * grp + 8):
                                nc.tensor.matmul(
                                    gps[:, (j - 8 * grp) * 16:
                                        (j - 8 * grp + 1) * 16],
                                    whh_sb[:, c_in * 4 * H + j * 128:
                                           c_in * 4 * H + (j + 1) * 128],
                                    h_prev[:, c_in * 16:(c_in + 1) * 16],
                                    start=False,
                                    stop=(c_in == 3 and j == 8 * grp + 7))
                        gh.append(gps)
                        if grp == 0:
                            tgg = qpool.tile([128, 64], F32, tag="tgg")
                            nc.scalar.activation(tgg[:], gh[0][:, 0:64],
                                                 AF.Tanh)
                            sgi = qpool.tile([128, 64], F32, tag="sgi")
                            nc.scalar.activation(sgi[:], gh[0][:, 64:128],
                                                 AF.Sigmoid)
                            tmp = qpool.tile([128, 64], F32, tag="tmp")
                            nc.vector.tensor_mul(tmp[:], sgi[:], tgg[:])
                    # stall fillers: previous step's emissions + deferred Xg
                    if t > 0:
                        em_ps = empsum.tile([K, BL], F32)
                        for c in range(4):
                            nc.tensor.matmul(
                                em_ps[:], wout_sb[:, c * K:(c + 1) * K],
                                h_prev[:, c * 16:(c + 1) * 16],
                                start=(c == 0), stop=(c == 3))
                        em_copies.append((t - 1, em_ps))
                    if t % 2 == 0 and xg_work:
                        emit_xg_unit(*xg_work.pop(0))

                    hn = qpool.tile([128, 64], BF16, tag="hn")
                    cn = qpool.tile([128, 64], F32, tag="cn")
                    sfo = qpool.tile([128, 128], F32, tag="sfo")
                    nc.scalar.activation(sfo[:], gh[1][:], AF.Sigmoid)
                    nc.vector.tensor_mul(cn[:], sfo[:, 0:64], c_prev[:])
                    nc.vector.tensor_add(cn[:], cn[:], tmp[:])
                    tc_sb = qpool.tile([128, 64], F32, tag="tc")
                    nc.scalar.activation(tc_sb[:], cn[:], AF.Tanh)
                    nc.vector.tensor_mul(hn[:], sfo[:, 64:128], tc_sb[:])
                    h_prev, c_prev = hn, cn
                    if em_copies:
                        te, eps = em_copies.pop()
                        nc.vector.tensor_copy(
                            em_sb[:, te * BL:(te + 1) * BL], eps[:])
                while xg_work:
                    emit_xg_unit(*xg_work.pop(0))
                em_ps = empsum.tile([K, BL], F32)
                for c in range(4):
                    nc.tensor.matmul(em_ps[:], wout_sb[:, c * K:(c + 1) * K],
                                     h_prev[:, c * 16:(c + 1) * 16],
                                     start=(c == 0), stop=(c == 3))
                nc.vector.tensor_copy(
                    em_sb[:, (TS - 1) * BL:TS * BL], em_ps[:])

        # ---------------- phase D: exchange + finalize emissions ------------
        with ExitStack() as d_stack:
            dpool = d_stack.enter_context(tc.tile_pool(name="dpool", bufs=1))
            # combine normal / time-reversed view by direction selector,
            # writing the result in b-major layout (col = bl*TS + t) so the
            # collective DMAs are contiguous
            cc_pre = dpool.tile([K, TB], F32)
            em_v = em_sb[:].rearrange("p (t b) -> p t b", t=TS)
            em_rv = em_v[:, ::-1, :]
            tmp_r = dpool.tile([K, TB], F32)
            tmp_r_bm = tmp_r[:].rearrange("p (b t) -> p t b", b=BL)
            cc_pre_bm = cc_pre[:].rearrange("p (b t) -> p t b", b=BL)
            nc.vector.tensor_scalar_mul(tmp_r_bm, em_rv, dirsel_sb[:, 1:2])
            nc.vector.scalar_tensor_tensor(
                cc_pre_bm, em_v, dirsel_sb[:, 0:1], tmp_r_bm,
                ALU.mult, ALU.add)
            for h in range(2):
                nc.sync.dma_start(
                    cc_in.ap()[32 * h:32 * h + 32, :],
                    cc_pre[:, 8 * h * TS:(8 * h + 8) * TS])
            nc.gpsimd.collective_compute(
                "ReduceScatter", ALU.add,
                ins=[cc_in.ap()], outs=[cc_out.ap()],
                replica_groups=[[0, 1], [2, 3], [4, 5], [6, 7]])
            em_fin = persist.tile([K, BT], F32)
            rs_sb = dpool.tile([K, BT], F32)
            nc.sync.dma_start(rs_sb[:], cc_out[:, :])
            nc.scalar.activation(em_fin[:], rs_sb[:], AF.Identity,
                                 bias=bout_sb[:, 0:1])
        expE = persist.tile([K, BT], F32)
        nc.scalar.activation(expE[:], em_fin[:], AF.Exp)

        # small tiles that cross the E/F phase boundary
        e_tot = persist.tile([1, BC], F32)
        t_tot = persist.tile([1, BC], F32)
        expT_sb = persist.tile([K, K], F32)
        expTs = persist.tile([K, 1], F32)
        expTe = persist.tile([K, 1], F32)
        k_acc = persist.tile([1, BC], I32)

        # ---------------- phase E: gold-path scores -------------------------
        with ExitStack() as e_stack:
            epool = e_stack.enter_context(tc.tile_pool(name="epool", bufs=2))
            epsum = e_stack.enter_context(
                tc.tile_pool(name="epsum", bufs=2, space="PSUM"))
            cpsum = e_stack.enter_context(
                tc.tile_pool(name="cpsum", bufs=1, space="PSUM"))

            # transition counts over extended sequences
            C_ps = cpsum.tile([K, BC * K], F32)
            chunk_starts = list(range(0, NPAIR, 128))
            for b in range(BC):
                for ci, s0 in enumerate(chunk_starts):
                    sz = min(128, NPAIR - s0)
                    tp = epool.tile([128, 1], F32, tag="tp")
                    nc.sync.dma_start(tp[:sz, :],
                                      tags_ext[b:b + 1, s0:s0 + sz])
                    tn = epool.tile([128, 1], F32, tag="tn")
                    nc.sync.dma_start(tn[:sz, :],
                                      tags_ext[b:b + 1, s0 + 1:s0 + 1 + sz])
                    ohp = epool.tile([128, K], F32, tag="ohp")
                    nc.vector.tensor_scalar(ohp[:sz, :], iota_row_sb[:sz, :],
                                            tp[:sz, :], None, ALU.is_equal)
                    ohn = epool.tile([128, K], F32, tag="ohn")
                    nc.vector.tensor_scalar(ohn[:sz, :], iota_row_sb[:sz, :],
                                            tn[:sz, :], None, ALU.is_equal)
                    nc.tensor.matmul(C_ps[:, b * K:(b + 1) * K],
                                     ohp[:sz, :], ohn[:sz, :],
                                     start=(ci == 0),
                                     stop=(ci == len(chunk_starts) - 1))
            trans8 = epool.tile([K, BC * K], F32, tag="trans8")
            for b in range(BC):
                nc.vector.tensor_copy(trans8[:, b * K:(b + 1) * K], trans_sb[:])
            tcmul = epool.tile([K, BC * K], F32, tag="tcmul")
            nc.vector.tensor_mul(tcmul[:], C_ps[:], trans8[:])
            tred = epool.tile([K, BC], F32, tag="tred")
            nc.vector.tensor_reduce(
                tred[:], tcmul[:].rearrange("p (b k) -> p b k", b=BC),
                mybir.AxisListType.X, ALU.add)
            ttot_ps = cpsum.tile([1, BC], F32, tag="ttot")
            nc.tensor.matmul(ttot_ps[:], ones32[:], tred[:],
                             start=True, stop=True)
            nc.vector.tensor_copy(t_tot[:], ttot_ps[:])

            # emission scores: one-hot mask + partition sum + t-reduction
            NSL = min(512, BT)
            for s in range(BT // NSL):
                sl = slice(s * NSL, (s + 1) * NSL)
                tb_ps = epsum.tile([K, NSL], F32, tag="tbps")
                nc.tensor.matmul(tb_ps[:], ones1x32[:], tagsflat_sb[:, sl],
                                 start=True, stop=True)
                ohm = epool.tile([K, NSL], F32, tag="ohm")
                nc.vector.tensor_scalar(ohm[:], tb_ps[:], iota_kp_sb[:],
                                        None, ALU.is_equal)
                nc.vector.tensor_mul(ohm[:], ohm[:], em_fin[:, sl])
                es_ps = epsum.tile([1, NSL], F32, tag="esps")
                nc.tensor.matmul(es_ps[:], ones32[:], ohm[:],
                                 start=True, stop=True)
                nb = NSL // TS
                nc.vector.tensor_reduce(
                    e_tot[:, s * nb:(s + 1) * nb],
                    es_ps[:].rearrange("p (b t) -> p b t", t=TS),
                    mybir.AxisListType.X, ALU.add)

        # ------------- phase F: CRF forward recurrence ------------------
        with ExitStack() as f_stack:
            fpool = f_stack.enter_context(tc.tile_pool(name="fpool", bufs=2))
            fpsum = f_stack.enter_context(
                tc.tile_pool(name="fpsum", bufs=2, space="PSUM"))

            nc.scalar.activation(expT_sb[:], trans_sb[:], AF.Exp)
            nc.scalar.activation(expTs[:], transT_sb[:, START:START + 1],
                                 AF.Exp)
            nc.scalar.activation(expTe[:], trans_sb[:, END:END + 1], AF.Exp)

            expE_v = expE[:].rearrange("p (b t) -> p b t", b=BC)
            a_cur = fpool.tile([K, BC], F32, tag="a")
            nc.vector.tensor_scalar_mul(a_cur[:], expE_v[:, :, 0], expTs[:])
            nc.vector.memset(k_acc[:], 0)

            for t in range(1, TS):
                a_ps = fpsum.tile([K, BC], F32, tag="aps")
                nc.tensor.matmul(a_ps[:], expT_sb[:], a_cur[:],
                                 start=True, stop=True)
                a_nxt = fpool.tile([K, BC], F32, tag="a")
                nc.vector.tensor_mul(a_nxt[:], a_ps[:], expE_v[:, :, t])
                a_cur = a_nxt
                if t % 8 == 0:
                    zps_t = fpsum.tile([K, BC], F32, tag="fps")
                    z_ps = zps_t[0:1, :]
                    nc.tensor.matmul(z_ps[:], ones32[:], a_cur[:],
                                     start=True, stop=True)
                    z_sb = fpool.tile([1, BC], F32, tag="zsb")
                    nc.vector.tensor_copy(z_sb[:], z_ps[:])
                    e_i = fpool.tile([1, BC], I32, tag="ei")
                    nc.vector.tensor_scalar(e_i[:], z_sb[:].bitcast(I32),
                                            23, None,
                                            ALU.logical_shift_right)
                    nc.vector.tensor_add(k_acc[:], k_acc[:], e_i[:])
                    sc_i = fpool.tile([1, BC], I32, tag="sci")
                    nc.vector.tensor_scalar(sc_i[:], e_i[:], -1, 254,
                                            ALU.mult, ALU.add)
                    nc.vector.tensor_scalar(sc_i[:], sc_i[:], 23, None,
                                            ALU.logical_shift_left)
                    bc_ps = fpsum.tile([K, BC], F32, tag="fps")
                    nc.tensor.matmul(bc_ps[:], ones1x32[:],
                                     sc_i[:].bitcast(F32),
                                     start=True, stop=True)
                    a_sc = fpool.tile([K, BC], F32, tag="a")
                    nc.vector.tensor_mul(a_sc[:], a_cur[:], bc_ps[:])
                    a_cur = a_sc

            zf_t = fpsum.tile([K, BC], F32, tag="fps")
            zf_ps = zf_t[0:1, :]
            nc.tensor.matmul(zf_ps[:], expTe[:], a_cur[:],
                             start=True, stop=True)
            logz = fpool.tile([1, BC], F32, tag="logz")
            nc.scalar.activation(logz[:], zf_ps[:], AF.Ln)
            k_f = fpool.tile([1, BC], F32, tag="kf")
            nc.vector.tensor_copy(k_f[:], k_acc[:])
            # nll = logz + ln2*(sum e) - 127*ln2*n_ev - e_tot - t_tot
            nll = fpool.tile([1, BC], F32, tag="nll")
            nc.vector.tensor_scalar(nll[:], k_f[:], LN2,
                                    -127.0 * LN2 * N_EV, ALU.mult, ALU.add)
            nc.vector.tensor_add(nll[:], nll[:], logz[:])
            nc.vector.tensor_sub(nll[:], nll[:], e_tot[:])
            nc.vector.tensor_sub(nll[:], nll[:], t_tot[:])
            nc.sync.dma_start(out[:, :], nll[:])


# ---------------------------------------------------------------------------
# host side
# ---------------------------------------------------------------------------
def _perm_rows(W):
    # gate-major blocks reordered g,i,f,o (pytorch order is i,f,g,o)
    out = np.empty_like(W)
    out[0:512] = W[1024:1536]        # g
    out[512:1024] = W[0:512]         # i
    out[1024:1536] = W[512:1024]     # f
    out[1536:2048] = W[1536:2048]    # o
    return out


def make_in_maps(inputs, t_steps=T):
    TS = t_steps
    X = np.asarray(inputs['X'], np.float32)
    tags = np.asarray(inputs['tags']).astype(np.int64)
    W = {d: (np.asarray(inputs[f'W_ih_{d}'], np.float32),
             np.asarray(inputs[f'W_hh_{d}'], np.float32),
             np.asarray(inputs[f'b_ih_{d}'], np.float32)
             + np.asarray(inputs[f'b_hh_{d}'], np.float32))
         for d in ('f', 'b')}
    W_out = np.asarray(inputs['W_out'], np.float32)
    b_out = np.asarray(inputs['b_out'], np.float32)
    trans = np.asarray(inputs['transitions'], np.float32)

    iota_row = np.tile(np.arange(K, dtype=np.float32), (128, 1))
    iota_kp = np.arange(K, dtype=np.float32)[:, None]

    maps = []
    for c in range(N_CORES):
        d = 'f' if c % 2 == 0 else 'b'
        w = c // 2
        b0 = BL * w
        Wih, Whh, bsum = W[d]
        wihT = _perm_rows(Wih).T.astype(ml_dtypes.bfloat16)      # [E, 4H]
        whhT = _perm_rows(Whh).T.astype(ml_dtypes.bfloat16)      # [H, 4H]
        biasT = _perm_rows(bsum[:, None])[:, 0].reshape(16, 128).T.copy()
        wo = W_out[(0 if d == 'f' else H):(H if d == 'f' else 2 * H), :]
        Xs = X[b0:b0 + BL, :TS, :]                               # [BL, TS, E]
        XT = Xs.transpose(2, 1, 0)                               # [E, TS, BL]
        if d == 'b':
            XT = XT[:, ::-1, :]
        crf = tags[b0 + (0 if d == 'f' else BC):
                   b0 + (BC if d == 'f' else 2 * BC), :TS]
        text = np.concatenate(
            [np.full((BC, 1), START), crf, np.full((BC, 1), END)],
            1).astype(np.float32)
        maps.append({
            "xT": np.ascontiguousarray(
                XT.reshape(2, 128, TS * BL)).astype(ml_dtypes.bfloat16),
            "wihT": np.ascontiguousarray(wihT.reshape(2, 128, 4 * H)),
            "whhT": np.ascontiguousarray(whhT.reshape(4, 128, 4 * H)),
            "biasT": np.ascontiguousarray(biasT).astype(np.float32),
            "woutT": np.ascontiguousarray(
                wo.reshape(4, 128, K)).astype(ml_dtypes.bfloat16),
            "bout": b_out[:, None].astype(np.float32),
            "trans": trans,
            "transT": np.ascontiguousarray(trans.T),
            "dirsel": np.tile(np.float32([1.0, 0.0] if d == 'f' else [0.0, 1.0]),
                              (K, 1)).astype(np.float32),
            "tags_ext": text,
            "tags_flat": crf.reshape(1, -1).astype(np.float32),
            "iota_row": iota_row,
            "iota_kp": iota_kp,
            "ident": np.eye(128, dtype=ml_dtypes.bfloat16),
        })
    return maps


def assemble_out(results):
    nll = np.zeros(B, np.float32)
    for c in range(N_CORES):
        w = c // 2
        off = 16 * w + (0 if c % 2 == 0 else BC)
        nll[off:off + BC] = results[c]["out"][0]
    return nll


_CACHED = {}


def kernel(**inputs):
    masks = np.asarray(inputs['masks'], np.float32)
    assert np.all(masks == 1.0), "kernel assumes masks == 1 (setup_inputs)"
    if 'nc' not in _CACHED:
        nc = build_nc()
        _split_multiwait(nc)
        _CACHED['nc'] = nc
    in_maps = make_in_maps(inputs)
    res = run_bass_kernel_spmd(_CACHED['nc'], in_maps,
                               core_ids=list(range(N_CORES)))
    return assemble_out(res.results)


# revision 52
# speedup vs baseline: 1.1740x; 1.1740x over previous
"""BiLSTM-CRF loss kernel for 8 Trainium2 NeuronCores.

Sharding: direction x batch. Even cores run the forward LSTM, odd cores the
backward LSTM (on host-time-reversed input). Core pair (2w, 2w+1) owns batch
window [16w, 16w+16). Each core computes its direction's partial emissions
(W_out matmul fused into the recurrence), the pair exchanges partials with one
ReduceScatter, and each core then runs the CRF (factored exp-space recurrence:
one 32x32 matmul + one elementwise multiply per step, with power-of-2
renormalization every 8 steps) plus the gold-path score (one-hot / transition
count-matrix matmuls) for 8 batches, producing nll[8].

Self-contained: hardcodes all shapes; no sibling imports.
"""

import numpy as np
import ml_dtypes

import concourse.bass as bass
import concourse.tile as tile
from concourse import mybir
from concourse.tile import add_dep_helper
from concourse.bass_utils import run_bass_kernel_spmd

F32 = mybir.dt.float32
BF16 = mybir.dt.bfloat16
I32 = mybir.dt.int32
AF = mybir.ActivationFunctionType
ALU = mybir.AluOpType

N_CORES = 8
B, T, E, H, K = 64, 256, 256, 512, 32
START, END = 30, 31
BL = 16   # batch per LSTM core
BC = 8    # batch per CRF core
LN2 = float(np.log(2.0))


# ---------------------------------------------------------------------------
# walrus-compat: this container's walrus supports only ONE sync-wait per
# instruction; Tile sometimes emits more. Split extras onto same-engine NOPs
# inserted just before the offending instruction.
# ---------------------------------------------------------------------------
def _split_multiwait(nc):
    import bass_rust
    n = 0
    for f in nc.m.functions:
        for bb in f.blocks:
            insts = bb.instructions
            if not insts:
                continue
            out = []
            changed = False
            for ins in insts:
                si = ins.sync_info
                if si is not None and si.on_wait and len(si.on_wait) > 1:
                    waits = list(si.on_wait)
                    eng = nc.engines[ins.engine]
                    for w in waits[:-1]:
                        nop = eng.nop()
                        nop_ins = nop.ins
                        cur_list = nc.cur_bb.bb.instructions
                        assert cur_list and cur_list[-1].name == nop_ins.name
                        cur_list.pop()
                        nop_ins.sync_info = bass_rust.SyncInfo(
                            on_wait=[w], on_update=[]
                        )
                        out.append(nop_ins)
                        n += 1
                    si.on_wait = [waits[-1]]
                    ins.sync_info = si
                    changed = True
                out.append(ins)
            if changed:
                bb.instructions = out
    return n


# ---------------------------------------------------------------------------
# Strip per-matmul completion increments. Every MATMUL increments the PE
# semaphore at completion and these EVT_SEM writes serialize (~26ns each), so
# the completion counter lags issue and everything waiting on "group
# complete" stalls. Keep only the increments whose cumulative value some wait
# actually targets and remap all thresholds.
# ---------------------------------------------------------------------------
def _strip_mm_incs(nc):
    blocks = [bb for f in nc.m.functions for bb in f.blocks]
    mm_sems = set()
    for bb in blocks:
        for ins in bb.instructions:
            si = ins.sync_info
            if si is None or not si.on_update:
                continue
            if type(ins).__name__ == 'InstMatmult':
                for u in si.on_update:
                    if u.update_mode == 'sem-inc':
                        mm_sems.add(u.id)
    stripped = 0
    for sem in mm_sems:
        targets = set()
        for bb in blocks:
            for ins in bb.instructions:
                si = ins.sync_info
                if si is None:
                    continue
                for w in (si.on_wait or []):
                    if w.id == sem and w.wait_mode == 'sem-ge-imm':
                        targets.add(w.wait_value)
        cum = 0
        keep_cum = []
        for bb in blocks:
            for ins in bb.instructions:
                si = ins.sync_info
                if si is None or not si.on_update:
                    continue
                ups = list(si.on_update)
                new_ups = []
                for u in ups:
                    if u.id != sem or u.update_mode != 'sem-inc':
                        new_ups.append(u)
                        continue
                    assert u.update_value == 1
                    cum += 1
                    if type(ins).__name__ == 'InstMatmult' and \
                            cum not in targets:
                        stripped += 1
                    else:
                        keep_cum.append(cum)
                        new_ups.append(u)
                if len(new_ups) != len(ups):
                    si.on_update = new_ups
                    ins.sync_info = si
        import bisect
        for bb in blocks:
            for ins in bb.instructions:
                si = ins.sync_info
                if si is None or not si.on_wait:
                    continue
                ch = False
                ws = list(si.on_wait)
                for w in ws:
                    if w.id == sem and w.wait_mode == 'sem-ge-imm':
                        nv = bisect.bisect_right(keep_cum, w.wait_value)
                        if nv != w.wait_value:
                            w.wait_value = nv
                            ch = True
                if ch:
                    si.on_wait = ws
                    ins.sync_info = si
    return stripped


# ---------------------------------------------------------------------------
# device program
# ---------------------------------------------------------------------------
def build_nc(t_steps=T, n_cores=N_CORES):
    TS = t_steps
    TB = BL * TS           # (t, b) columns per LSTM core
    BT = BC * TS           # (b, t) columns per CRF core (b-major)
    NPAIR = TS + 1         # transition pairs incl. START->t0 and tlast->END
    N_EV = (TS - 1) // 8   # renorm events

    nc = bass.Bass("TRN2", target_bir_lowering=False, debug=False,
                   num_devices=n_cores)

    # inputs (all staged per-core on host)
    xT = nc.dram_tensor("xT", [2, 128, TB], BF16, kind="ExternalInput")
    wihT = nc.dram_tensor("wihT", [2, 128, 4 * H], BF16, kind="ExternalInput")
    whhT = nc.dram_tensor("whhT", [4, 128, 4 * H], BF16, kind="ExternalInput")
    biasT = nc.dram_tensor("biasT", [128, 16], F32, kind="ExternalInput")
    woutT = nc.dram_tensor("woutT", [4, 128, K], BF16, kind="ExternalInput")
    bout = nc.dram_tensor("bout", [K, 1], F32, kind="ExternalInput")
    trans = nc.dram_tensor("trans", [K, K], F32, kind="ExternalInput")
    transT = nc.dram_tensor("transT", [K, K], F32, kind="ExternalInput")
    dirsel = nc.dram_tensor("dirsel", [K, 2], F32, kind="ExternalInput")
    NCH_E = (TS + 1 + 127) // 128
    tags_ext = nc.dram_tensor("tags_ext", [128, BC * NCH_E * 2], F32,
                              kind="ExternalInput")
    tags_flat = nc.dram_tensor("tags_flat", [1, BT], F32, kind="ExternalInput")
    iota_row = nc.dram_tensor("iota_row", [128, K], F32, kind="ExternalInput")
    iota_kp = nc.dram_tensor("iota_kp", [K, 1], F32, kind="ExternalInput")
    ident = nc.dram_tensor("ident", [128, 128], BF16, kind="ExternalInput")
    out = nc.dram_tensor("out", [1, BC], F32, kind="ExternalOutput")

    # collective bounce buffers
    cc_in = nc.dram_tensor("cc_in", [2 * K, BT], F32)
    cc_out = nc.dram_tensor("cc_out", [K, BT], F32)

    with tile.TileContext(nc) as tc:
        _body(tc, locals(), TS, TB, BT, NPAIR, N_EV)
    _strip_mm_incs(nc)
    return nc


def _body(tc, io, TS, TB, BT, NPAIR, N_EV):
    from contextlib import ExitStack
    nc = tc.nc
    xT, wihT, whhT, biasT, woutT = io['xT'], io['wihT'], io['whhT'], io['biasT'], io['woutT']
    bout, trans, transT, dirsel = io['bout'], io['trans'], io['transT'], io['dirsel']
    tags_ext, tags_flat, iota_row, iota_kp = io['tags_ext'], io['tags_flat'], io['iota_row'], io['iota_kp']
    ident = io['ident']
    out, cc_in, cc_out = io['out'], io['cc_in'], io['cc_out']

    with ExitStack() as top:
        persist = top.enter_context(tc.tile_pool(name="persist", bufs=1))

        # persistent tiles
        em_sb = persist.tile([K, TB], F32)           # partial emissions (t,b)
        bias_sb = persist.tile([128, 16], F32)
        nc.sync.dma_start(bias_sb[:], biasT[:, :])
        trans_sb = persist.tile([K, K], F32)
        nc.sync.dma_start(trans_sb[:], trans[:, :])
        transT_sb = persist.tile([K, K], F32)
        nc.sync.dma_start(transT_sb[:], transT[:, :])
        dirsel_sb = persist.tile([K, 2], F32)
        nc.sync.dma_start(dirsel_sb[:], dirsel[:, :])
        bout_sb = persist.tile([K, 1], F32)
        nc.sync.dma_start(bout_sb[:], bout[:, :])
        iota_row_sb = persist.tile([128, K], F32)
        nc.sync.dma_start(iota_row_sb[:], iota_row[:, :])
        iota_kp_sb = persist.tile([K, 1], F32)
        nc.sync.dma_start(iota_kp_sb[:], iota_kp[:, :])
        tagsflat_sb = persist.tile([1, BT], F32)
        nc.sync.dma_start(tagsflat_sb[:], tags_flat[:, :])
        ones32 = persist.tile([K, 1], F32)
        nc.vector.memset(ones32[:], 1.0)
        ones1x32 = persist.tile([1, K], F32)
        nc.vector.memset(ones1x32[:], 1.0)
        ident_sb = persist.tile([128, 128], BF16)
        nc.sync.dma_start(ident_sb[:], ident[:, :])

        # ---------------- phase BC pool (xg + recurrence state) -------------
        with ExitStack() as bc_stack:
            bcpool = bc_stack.enter_context(tc.tile_pool(name="bcpool", bufs=1))
            xg_sb = bcpool.tile([128, 16 * TB], BF16)

            # ---------------- phase B: Xg = W_ih_p @ X^T + bias -------------
            if True:
                bpool = bc_stack.enter_context(tc.tile_pool(name="bpool", bufs=1))
                bxpool = bc_stack.enter_context(tc.tile_pool(name="bxpool", bufs=2))
                bpsum = bc_stack.enter_context(
                    tc.tile_pool(name="bpsum", bufs=2, space="PSUM"))
                wi0 = bpool.tile([128, 4 * H], BF16)
                nc.sync.dma_start(wi0[:], wihT[0, :, :])
                wi1 = bpool.tile([128, 4 * H], BF16)
                nc.sync.dma_start(wi1[:], wihT[1, :, :])
                NCH = min(512, TB)
                NN = TB // NCH
                xs_cur = {}

                def load_x_chunk(n):
                    xs0 = bxpool.tile([128, NCH], BF16, tag="xs0")
                    nc.sync.dma_start(xs0[:], xT[0, :, n * NCH:(n + 1) * NCH])
                    xs1 = bxpool.tile([128, NCH], BF16, tag="xs1")
                    nc.sync.dma_start(xs1[:], xT[1, :, n * NCH:(n + 1) * NCH])
                    xs_cur[0], xs_cur[1] = xs0, xs1

                def emit_xg_unit(j, n):
                    if j == 0:
                        load_x_chunk(n)
                    ps = bpsum.tile([128, NCH], F32, tag="bps")
                    nc.tensor.matmul(ps[:], wi0[:, j * 128:(j + 1) * 128],
                                     xs_cur[0][:], start=True, stop=False)
                    nc.tensor.matmul(ps[:], wi1[:, j * 128:(j + 1) * 128],
                                     xs_cur[1][:], start=False, stop=True)
                    dst = xg_sb[:, j * TB + n * NCH: j * TB + (n + 1) * NCH]
                    if (j + n) % 2 == 0:
                        nc.scalar.activation(dst, ps[:], AF.Identity,
                                             bias=bias_sb[:, j:j + 1])
                    else:
                        nc.vector.tensor_scalar_add(dst, ps[:],
                                                    bias_sb[:, j:j + 1])

                # prologue: first t-chunk (n=0) of Xg for every j; the rest is
                # interleaved into the recurrence to fill PE stalls
                xg_work = []
                for n in range(NN):
                    for j in range(16):
                        if n == 0:
                            emit_xg_unit(j, n)
                        else:
                            xg_work.append((j, n))

            # ---------------- phase C: recurrence + fused emissions ---------
            with ExitStack() as c_stack:
                cpool = c_stack.enter_context(tc.tile_pool(name="cpool", bufs=1))
                whh_sb = cpool.tile([128, 4 * 4 * H], BF16)
                for c in range(4):
                    nc.sync.dma_start(
                        whh_sb[:, c * 4 * H:(c + 1) * 4 * H], whhT[c, :, :])
                wout_sb = cpool.tile([128, 4 * K], BF16)
                for c in range(4):
                    nc.sync.dma_start(wout_sb[:, c * K:(c + 1) * K],
                                      woutT[c, :, :])

                spool = c_stack.enter_context(tc.tile_pool(name="spool", bufs=2))
                qpool = c_stack.enter_context(tc.tile_pool(name="qpool", bufs=3))
                gpsum = c_stack.enter_context(
                    tc.tile_pool(name="gpsum", bufs=1, space="PSUM"))
                empsum = c_stack.enter_context(
                    tc.tile_pool(name="empsum", bufs=2, space="PSUM"))

                # single stream, half-split h/c; Xg preloaded into PSUM via
                # identity matmul so gates = PSUM directly (no DVE add)
                h_prev = spool.tile([128, 64], BF16, tag="h")
                nc.vector.memset(h_prev[:], 0.0)
                c_prev = spool.tile([128, 64], F32, tag="c")
                nc.vector.memset(c_prev[:], 0.0)

                xg_v = xg_sb[:].rearrange("p (j t b) -> p j t b", j=16, t=TS)
                em_copies = []
                for t in range(TS):
                    gps = gpsum.tile([128, 256], F32, tag=f"g{t % 2}",
                                     name="gps", bufs=2)
                    for gt in range(2):
                        nc.tensor.matmul(
                            gps[:, gt * 128:(gt + 1) * 128], ident_sb[:],
                            xg_v[:, 8 * gt:8 * gt + 8, t, :],
                            start=(gt == 0), stop=False)
                    for c_in in range(4):
                        for j in range(16):
                            nc.tensor.matmul(
                                gps[:, j * 16:(j + 1) * 16],
                                whh_sb[:, c_in * 4 * H + j * 128:
                                       c_in * 4 * H + (j + 1) * 128],
                                h_prev[:, c_in * 16:(c_in + 1) * 16],
                                start=False,
                                stop=(c_in == 3 and j == 15))
                    # stall fillers: previous step's emissions + deferred Xg
                    if t > 0:
                        em_ps = empsum.tile([K, BL], F32)
                        for c in range(4):
                            nc.tensor.matmul(
                                em_ps[:], wout_sb[:, c * K:(c + 1) * K],
                                h_prev[:, c * 16:(c + 1) * 16],
                                start=(c == 0), stop=(c == 3))
                        em_copies.append((t - 1, em_ps))
                    if t % 2 == 0 and xg_work:
                        emit_xg_unit(*xg_work.pop(0))

                    hn = qpool.tile([128, 64], BF16, tag="hn")
                    cn = qpool.tile([128, 64], F32, tag="cn")
                    sig = qpool.tile([128, 192], F32, tag="sig")
                    nc.scalar.activation(sig[:], gps[:, 0:192], AF.Sigmoid)
                    nc.vector.tensor_mul(cn[:], sig[:, 64:128], c_prev[:])
                    tg = qpool.tile([128, 64], F32, tag="tg")
                    nc.scalar.activation(tg[:], gps[:, 192:256], AF.Tanh)
                    tmp = qpool.tile([128, 64], F32, tag="tmp")
                    nc.vector.tensor_mul(tmp[:], sig[:, 0:64], tg[:])
                    nc.vector.tensor_add(cn[:], cn[:], tmp[:])
                    tc_sb = qpool.tile([128, 64], F32, tag="tc")
                    nc.scalar.activation(tc_sb[:], cn[:], AF.Tanh)
                    nc.vector.tensor_mul(hn[:], sig[:, 128:192], tc_sb[:])
                    h_prev, c_prev = hn, cn
                    if em_copies:
                        te, eps = em_copies.pop()
                        nc.vector.tensor_copy(
                            em_sb[:, te * BL:(te + 1) * BL], eps[:])
                while xg_work:
                    emit_xg_unit(*xg_work.pop(0))
                em_ps = empsum.tile([K, BL], F32)
                for c in range(4):
                    nc.tensor.matmul(em_ps[:], wout_sb[:, c * K:(c + 1) * K],
                                     h_prev[:, c * 16:(c + 1) * 16],
                                     start=(c == 0), stop=(c == 3))
                nc.vector.tensor_copy(
                    em_sb[:, (TS - 1) * BL:TS * BL], em_ps[:])

        # ---------------- phase D: exchange + finalize emissions ------------
        with ExitStack() as d_stack:
            dpool = d_stack.enter_context(tc.tile_pool(name="dpool", bufs=1))
            # combine normal / time-reversed view by direction selector,
            # writing the result in b-major layout (col = bl*TS + t) so the
            # collective DMAs are contiguous
            cc_pre = dpool.tile([K, TB], F32)
            em_v = em_sb[:].rearrange("p (t b) -> p t b", t=TS)
            em_rv = em_v[:, ::-1, :]
            tmp_r = dpool.tile([K, TB], F32)
            tmp_r_bm = tmp_r[:].rearrange("p (b t) -> p t b", b=BL)
            cc_pre_bm = cc_pre[:].rearrange("p (b t) -> p t b", b=BL)
            nc.vector.tensor_scalar_mul(tmp_r_bm, em_rv, dirsel_sb[:, 1:2])
            nc.vector.scalar_tensor_tensor(
                cc_pre_bm, em_v, dirsel_sb[:, 0:1], tmp_r_bm,
                ALU.mult, ALU.add)
            for h in range(2):
                nc.sync.dma_start(
                    cc_in.ap()[32 * h:32 * h + 32, :],
                    cc_pre[:, 8 * h * TS:(8 * h + 8) * TS])
            nc.gpsimd.collective_compute(
                "ReduceScatter", ALU.add,
                ins=[cc_in.ap()], outs=[cc_out.ap()],
                replica_groups=[[0, 1], [2, 3], [4, 5], [6, 7]])
            em_fin = persist.tile([K, BT], F32)
            rs_sb = dpool.tile([K, BT], F32)
            nc.sync.dma_start(rs_sb[:], cc_out[:, :])
            nc.scalar.activation(em_fin[:], rs_sb[:], AF.Identity,
                                 bias=bout_sb[:, 0:1])
        expE = persist.tile([K, BT], F32)
        nc.scalar.activation(expE[:], em_fin[:], AF.Exp)

        # small tiles that cross the E/F phase boundary
        e_tot = persist.tile([1, BC], F32)
        t_tot = persist.tile([1, BC], F32)
        expT_sb = persist.tile([K, K], F32)
        expTs = persist.tile([K, 1], F32)
        expTe = persist.tile([K, 1], F32)
        k_acc = persist.tile([1, BC], I32)

        # ---------------- phase E: gold-path scores -------------------------
        with ExitStack() as e_stack:
            epool = e_stack.enter_context(tc.tile_pool(name="epool", bufs=2))
            epsum = e_stack.enter_context(
                tc.tile_pool(name="epsum", bufs=2, space="PSUM"))
            cpsum = e_stack.enter_context(
                tc.tile_pool(name="cpsum", bufs=1, space="PSUM"))

            # transition counts over extended sequences; tag columns come in
            # one batched DMA (out-of-range rows hold -1 -> all-zero one-hot)
            NCH_E = (TS + 1 + 127) // 128
            tagsC = epool.tile([128, BC * NCH_E * 2], F32, tag="tagsC")
            nc.sync.dma_start(tagsC[:], tags_ext[:, :])
            C_ps = cpsum.tile([K, BC * K], F32)
            for b in range(BC):
                for ci in range(NCH_E):
                    col = (b * NCH_E + ci) * 2
                    ohp = epool.tile([128, K], F32, tag="ohp")
                    nc.vector.tensor_scalar(ohp[:], iota_row_sb[:],
                                            tagsC[:, col:col + 1],
                                            None, ALU.is_equal)
                    ohn = epool.tile([128, K], F32, tag="ohn")
                    nc.vector.tensor_scalar(ohn[:], iota_row_sb[:],
                                            tagsC[:, col + 1:col + 2],
                                            None, ALU.is_equal)
                    nc.tensor.matmul(C_ps[:, b * K:(b + 1) * K],
                                     ohp[:], ohn[:],
                                     start=(ci == 0),
                                     stop=(ci == NCH_E - 1))
            trans8 = epool.tile([K, BC * K], F32, tag="trans8")
            for b in range(BC):
                nc.vector.tensor_copy(trans8[:, b * K:(b + 1) * K], trans_sb[:])
            tcmul = epool.tile([K, BC * K], F32, tag="tcmul")
            nc.vector.tensor_mul(tcmul[:], C_ps[:], trans8[:])
            tred = epool.tile([K, BC], F32, tag="tred")
            nc.vector.tensor_reduce(
                tred[:], tcmul[:].rearrange("p (b k) -> p b k", b=BC),
                mybir.AxisListType.X, ALU.add)
            ttot_ps = cpsum.tile([1, BC], F32, tag="ttot")
            nc.tensor.matmul(ttot_ps[:], ones32[:], tred[:],
                             start=True, stop=True)
            nc.vector.tensor_copy(t_tot[:], ttot_ps[:])

            # emission scores: one-hot mask + partition sum + t-reduction
            NSL = min(512, BT)
            for s in range(BT // NSL):
                sl = slice(s * NSL, (s + 1) * NSL)
                tb_ps = epsum.tile([K, NSL], F32, tag="tbps")
                nc.tensor.matmul(tb_ps[:], ones1x32[:], tagsflat_sb[:, sl],
                                 start=True, stop=True)
                ohm = epool.tile([K, NSL], F32, tag="ohm")
                nc.vector.tensor_scalar(ohm[:], tb_ps[:], iota_kp_sb[:],
                                        None, ALU.is_equal)
                nc.vector.tensor_mul(ohm[:], ohm[:], em_fin[:, sl])
                es_ps = epsum.tile([1, NSL], F32, tag="esps")
                nc.tensor.matmul(es_ps[:], ones32[:], ohm[:],
                                 start=True, stop=True)
                nb = NSL // TS
                nc.vector.tensor_reduce(
                    e_tot[:, s * nb:(s + 1) * nb],
                    es_ps[:].rearrange("p (b t) -> p b t", t=TS),
                    mybir.AxisListType.X, ALU.add)

        # ------------- phase F: CRF forward recurrence ------------------
        with ExitStack() as f_stack:
            fpool = f_stack.enter_context(tc.tile_pool(name="fpool", bufs=2))
            fpsum = f_stack.enter_context(
                tc.tile_pool(name="fpsum", bufs=2, space="PSUM"))

            nc.scalar.activation(expT_sb[:], trans_sb[:], AF.Exp)
            nc.scalar.activation(expTs[:], transT_sb[:, START:START + 1],
                                 AF.Exp)
            nc.scalar.activation(expTe[:], trans_sb[:, END:END + 1], AF.Exp)

            expE_v = expE[:].rearrange("p (b t) -> p b t", b=BC)
            a_cur = fpool.tile([K, BC], F32, tag="a")
            nc.vector.tensor_scalar_mul(a_cur[:], expE_v[:, :, 0], expTs[:])
            nc.vector.memset(k_acc[:], 0)

            for t in range(1, TS):
                a_ps = fpsum.tile([K, BC], F32, tag="aps")
                nc.tensor.matmul(a_ps[:], expT_sb[:], a_cur[:],
                                 start=True, stop=True)
                a_nxt = fpool.tile([K, BC], F32, tag="a")
                nc.vector.tensor_mul(a_nxt[:], a_ps[:], expE_v[:, :, t])
                a_cur = a_nxt
                if t % 8 == 0:
                    zps_t = fpsum.tile([K, BC], F32, tag="fps")
                    z_ps = zps_t[0:1, :]
                    nc.tensor.matmul(z_ps[:], ones32[:], a_cur[:],
                                     start=True, stop=True)
                    z_sb = fpool.tile([1, BC], F32, tag="zsb")
                    nc.vector.tensor_copy(z_sb[:], z_ps[:])
                    e_i = fpool.tile([1, BC], I32, tag="ei")
                    nc.vector.tensor_scalar(e_i[:], z_sb[:].bitcast(I32),
                                            23, None,
                                            ALU.logical_shift_right)
                    nc.vector.tensor_add(k_acc[:], k_acc[:], e_i[:])
                    sc_i = fpool.tile([1, BC], I32, tag="sci")
                    nc.vector.tensor_scalar(sc_i[:], e_i[:], -1, 254,
                                            ALU.mult, ALU.add)
                    nc.vector.tensor_scalar(sc_i[:], sc_i[:], 23, None,
                                            ALU.logical_shift_left)
                    bc_ps = fpsum.tile([K, BC], F32, tag="fps")
                    nc.tensor.matmul(bc_ps[:], ones1x32[:],
                                     sc_i[:].bitcast(F32),
                                     start=True, stop=True)
                    a_sc = fpool.tile([K, BC], F32, tag="a")
                    nc.vector.tensor_mul(a_sc[:], a_cur[:], bc_ps[:])
                    a_cur = a_sc

            zf_t = fpsum.tile([K, BC], F32, tag="fps")
            zf_ps = zf_t[0:1, :]
            nc.tensor.matmul(zf_ps[:], expTe[:], a_cur[:],
                             start=True, stop=True)
            logz = fpool.tile([1, BC], F32, tag="logz")
            nc.scalar.activation(logz[:], zf_ps[:], AF.Ln)
            k_f = fpool.tile([1, BC], F32, tag="kf")
            nc.vector.tensor_copy(k_f[:], k_acc[:])
            # nll = logz + ln2*(sum e) - 127*ln2*n_ev - e_tot - t_tot
            nll = fpool.tile([1, BC], F32, tag="nll")
            nc.vector.tensor_scalar(nll[:], k_f[:], LN2,
                                    -127.0 * LN2 * N_EV, ALU.mult, ALU.add)
            nc.vector.tensor_add(nll[:], nll[:], logz[:])
            nc.vector.tensor_sub(nll[:], nll[:], e_tot[:])
            nc.vector.tensor_sub(nll[:], nll[:], t_tot[:])
            nc.sync.dma_start(out[:, :], nll[:])


# ---------------------------------------------------------------------------
# host side
# ---------------------------------------------------------------------------
def _perm_rows(W):
    # gate-major blocks reordered i,f,o,g (pytorch order is i,f,g,o)
    out = np.empty_like(W)
    out[0:1024] = W[0:1024]          # i, f
    out[1024:1536] = W[1536:2048]    # o
    out[1536:2048] = W[1024:1536]    # g
    return out


def make_in_maps(inputs, t_steps=T):
    TS = t_steps
    X = np.asarray(inputs['X'], np.float32)
    tags = np.asarray(inputs['tags']).astype(np.int64)
    W = {d: (np.asarray(inputs[f'W_ih_{d}'], np.float32),
             np.asarray(inputs[f'W_hh_{d}'], np.float32),
             np.asarray(inputs[f'b_ih_{d}'], np.float32)
             + np.asarray(inputs[f'b_hh_{d}'], np.float32))
         for d in ('f', 'b')}
    W_out = np.asarray(inputs['W_out'], np.float32)
    b_out = np.asarray(inputs['b_out'], np.float32)
    trans = np.asarray(inputs['transitions'], np.float32)

    iota_row = np.tile(np.arange(K, dtype=np.float32), (128, 1))
    iota_kp = np.arange(K, dtype=np.float32)[:, None]

    maps = []
    for c in range(N_CORES):
        d = 'f' if c % 2 == 0 else 'b'
        w = c // 2
        b0 = BL * w
        Wih, Whh, bsum = W[d]
        wihT = _perm_rows(Wih).T.astype(ml_dtypes.bfloat16)      # [E, 4H]
        whhT = _perm_rows(Whh).T.astype(ml_dtypes.bfloat16)      # [H, 4H]
        biasT = _perm_rows(bsum[:, None])[:, 0].reshape(16, 128).T.copy()
        wo = W_out[(0 if d == 'f' else H):(H if d == 'f' else 2 * H), :]
        Xs = X[b0:b0 + BL, :TS, :]                               # [BL, TS, E]
        XT = Xs.transpose(2, 1, 0)                               # [E, TS, BL]
        if d == 'b':
            XT = XT[:, ::-1, :]
        crf = tags[b0 + (0 if d == 'f' else BC):
                   b0 + (BC if d == 'f' else 2 * BC), :TS]
        text = np.concatenate(
            [np.full((BC, 1), START), crf, np.full((BC, 1), END)],
            1).astype(np.float32)
        # batched tag columns: col (b, chunk, prev/next), row r ->
        # ext[b, chunk*128 + r (+1 for next)], -1 past the end
        nch = (TS + 1 + 127) // 128
        tagsC = np.full((128, BC * nch * 2), -1.0, np.float32)
        npair = TS + 1
        for b in range(BC):
            for ci in range(nch):
                lo = ci * 128
                sz = min(128, npair - lo)
                if sz <= 0:
                    continue
                tagsC[:sz, (b * nch + ci) * 2] = text[b, lo:lo + sz]
                tagsC[:sz, (b * nch + ci) * 2 + 1] = text[b, lo + 1:lo + 1 + sz]
        maps.append({
            "xT": np.ascontiguousarray(
                XT.reshape(2, 128, TS * BL)).astype(ml_dtypes.bfloat16),
            "wihT": np.ascontiguousarray(wihT.reshape(2, 128, 4 * H)),
            "whhT": np.ascontiguousarray(whhT.reshape(4, 128, 4 * H)),
            "biasT": np.ascontiguousarray(biasT).astype(np.float32),
            "woutT": np.ascontiguousarray(
                wo.reshape(4, 128, K)).astype(ml_dtypes.bfloat16),
            "bout": b_out[:, None].astype(np.float32),
            "trans": trans,
            "transT": np.ascontiguousarray(trans.T),
            "dirsel": np.tile(np.float32([1.0, 0.0] if d == 'f' else [0.0, 1.0]),
                              (K, 1)).astype(np.float32),
            "tags_ext": tagsC,
            "tags_flat": crf.reshape(1, -1).astype(np.float32),
            "iota_row": iota_row,
            "iota_kp": iota_kp,
            "ident": np.eye(128, dtype=ml_dtypes.bfloat16),
        })
    return maps


def assemble_out(results):
    nll = np.zeros(B, np.float32)
    for c in range(N_CORES):
        w = c // 2
        off = 16 * w + (0 if c % 2 == 0 else BC)
        nll[off:off + BC] = results[c]["out"][0]
    return nll


_CACHED = {}


def kernel(**inputs):
    masks = np.asarray(inputs['masks'], np.float32)
    assert np.all(masks == 1.0), "kernel assumes masks == 1 (setup_inputs)"
    if 'nc' not in _CACHED:
        nc = build_nc()
        _split_multiwait(nc)
        _CACHED['nc'] = nc
    in_maps = make_in_maps(inputs)
    res = run_bass_kernel_spmd(_CACHED['nc'], in_maps,
                               core_ids=list(range(N_CORES)))
    return assemble_out(res.results)
